# revision 1
# baseline (speedup 1.0000x reference)
"""Int4-packed linear (group-quantized, 256-group) on 8 Trainium2 cores.

Column-parallel: each core owns 1024 of 8192 out_features.

Math per core (out^T orientation, o on partitions):
  out[t, o] = sum_g s[o,g] * R_g[o,t] - 8*sum_g s[o,g]*xsum_g[t] + bias[o]
  R_g[o,t]  = sum_{i in g} q[o,i] * x[t,i]        (q in 0..15)

Weights ship as fp8e4m3 nibble planes (exact small integers), x as bf16.
Group partials accumulate in PSUM slices; -8 offset + bias ride a tiny fp32
correction matmul into group 31's slice (pre-divided by s[:,31] so the
group-31 scale multiply restores it). Scale multiply = one DVE tensor_tensor
per o-tile with a step-0 broadcast AP; group reduction = GPSIMD tree adds.
"""

import sys

import numpy as np
import ml_dtypes

sys.path.insert(0, "/opt/trn_rl_repo")

import concourse.bass as bass  # noqa: E402
import concourse.mybir as mybir  # noqa: E402
import concourse.tile as tile  # noqa: E402
from concourse import bacc  # noqa: E402

NCORES = 8
TOKENS = 64
IN_F = 8192
OUT_F = 8192
GROUP = 256
OC = OUT_F // NCORES  # 1024 out-features per core
NCHUNK = IN_F // 128  # 64 K-chunks of 128
NG = IN_F // GROUP  # 32 groups
NOT = OC // 128  # 8 o-tiles per core

_cache = {}


def _raw_scan(nc, out, data0, data1, initial, op0, op1):
    """tensor_tensor_scan without the 2D-shape asserts. Multi-free-dim APs
    chain the recurrence across slice boundaries -- intended here: the g=0
    multiplier is 0, cutting the carry at each token boundary."""
    eng = nc.vector
    return eng.add_instruction(
        mybir.InstTensorScalarPtr(
            name=nc.get_next_instruction_name(),
            is_tensor_tensor_scan=True,
            is_scalar_tensor_tensor=True,
            op0=op0,
            op1=op1,
            ins=[
                eng.lower_ap(data0),
                eng.lower_ap_or_imm(initial),
                eng.lower_ap(data1),
            ],
            outs=[eng.lower_ap(out)],
        )
    )


def _build_nc():
    if "nc" in _cache:
        return _cache["nc"], _cache["names"]

    f32 = mybir.dt.float32
    nc = bacc.Bacc(None, target_bir_lowering=False, debug=False)
    with tile.TileContext(nc) as tc:
        with tc.tile_pool(name="dram", bufs=1, space="DRAM") as dram:
            w8 = dram.tile([128, NCHUNK, OC], mybir.dt.float8e4, kind="ExternalInput")
            xt = dram.tile([128, NCHUNK, TOKENS], mybir.dt.bfloat16, kind="ExternalInput")
            sc = dram.tile([128, NOT, NG], f32, kind="ExternalInput")
            # u[p, ot, g] = s[o,g-1]/s[o,g], 0 at g=0 (Horner ratio chain)
            ur = dram.tile([128, NOT, NG], f32, kind="ExternalInput")
            cl = dram.tile([NG + 1, OC], f32, kind="ExternalInput")
            cr = dram.tile([NG + 1, TOKENS], f32, kind="ExternalInput")
            outT = dram.tile([OC, TOKENS], f32, kind="ExternalOutput")

            with (
                tc.tile_pool(name="wsb", bufs=1) as wsb,
                tc.tile_pool(name="xsb", bufs=1) as xsb,
                tc.tile_pool(name="small", bufs=1) as small,
                tc.tile_pool(name="rs", bufs=3) as rspool,
                tc.tile_pool(name="yout", bufs=3) as ypool,
                tc.tile_pool(name="ps", bufs=2, space="PSUM") as ps,
            ):
                w_all = wsb.tile([128, NCHUNK, OC], mybir.dt.float8e4)
                x_all = xsb.tile([128, NCHUNK, TOKENS], mybir.dt.bfloat16)
                sc_all = small.tile([128, NOT, NG], f32)
                u_all = small.tile([128, NOT, NG], f32, tag="u")
                cl_sb = small.tile([NG + 1, OC], f32, tag="cl")
                cr_sb = small.tile([NG + 1, TOKENS], f32, tag="cr")

                # small operands first: every matmul needs x, so it must not
                # queue behind 8.4MB of weights on the HWDGE ring
                nc.sync.dma_start(out=x_all[:], in_=xt[:])
                nc.sync.dma_start(out=sc_all[:], in_=sc[:])
                nc.sync.dma_start(out=u_all[:], in_=ur[:])
                nc.sync.dma_start(out=cl_sb[:], in_=cl[:])
                nc.sync.dma_start(out=cr_sb[:], in_=cr[:])
                # weights in 8 chunk-blocks so matmuls start before all 8.4MB
                for b in range(8):
                    rsl = slice(b * 8, (b + 1) * 8)
                    nc.sync.dma_start(out=w_all[:, rsl, :], in_=w8[:, rsl, :])

                for ot in range(NOT):
                    osl = slice(ot * 128, (ot + 1) * 128)
                    r_ps = ps.tile([128, NG, TOKENS], f32)
                    for g in range(NG):
                        nc.tensor.matmul(
                            r_ps[:, g, :],
                            lhsT=w_all[:, 2 * g, osl],
                            rhs=x_all[:, 2 * g, :],
                            start=True,
                            stop=False,
                        )
                        nc.tensor.matmul(
                            r_ps[:, g, :],
                            lhsT=w_all[:, 2 * g + 1, osl],
                            rhs=x_all[:, 2 * g + 1, :],
                            start=False,
                            stop=(g != NG - 1),
                        )
                    # -8 offset + bias correction, pre-divided by s[:,31]
                    nc.tensor.matmul(
                        r_ps[:, NG - 1, :],
                        lhsT=cl_sb[:, osl],
                        rhs=cr_sb[:],
                        start=False,
                        stop=True,
                    )

                    # fused scale+reduce: Horner scan along g (t outer):
                    #   state(t,g) = u[g]*state + R[g]  with u[0]=0
                    #   => state(t,31) = sum_g R_g * s_g / s_31
                    u_ot = u_all[:, ot, :]
                    u_bcast = bass.AP(
                        tensor=u_ot.tensor,
                        offset=u_ot.offset,
                        ap=[u_ot.ap[0], [0, TOKENS], [1, NG]],
                    )
                    r_tg = bass.AP(
                        tensor=r_ps.tensor,
                        offset=r_ps.offset,
                        ap=[r_ps.ap[0], [1, TOKENS], [TOKENS, NG]],
                    )
                    rs = rspool.tile([128, TOKENS, NG], f32)
                    _raw_scan(
                        nc, rs[:], u_bcast, r_tg, 0.0,
                        mybir.AluOpType.mult, mybir.AluOpType.add,
                    )
                    # y[o, t] = state(t, 31) * s[o, 31]
                    y = ypool.tile([128, TOKENS], f32)
                    nc.vector.tensor_scalar(
                        out=y[:],
                        in0=rs[:, :, NG - 1],
                        scalar1=sc_all[:, ot, NG - 1 : NG],
                        scalar2=None,
                        op0=mybir.AluOpType.mult,
                    )
                    nc.sync.dma_start(out=outT[osl, :], in_=y[:])

    nc.compile()
    names = dict(w8=w8.name, xt=xt.name, sc=sc.name, ur=ur.name, cl=cl.name,
                 cr=cr.name, outT=outT.name)
    _cache["nc"] = nc
    _cache["names"] = names
    return nc, names


def _host_prep(x, weight_packed, scales, bias):
    """Build the 8 per-core input maps."""
    _, names = _build_nc()

    wp = np.ascontiguousarray(weight_packed).view(np.uint32)  # [8192, 1024]
    shifts = (np.arange(8, dtype=np.uint32) * 4)[None, None, :]
    nib = ((wp[:, :, None] >> shifts) & np.uint32(0xF)).astype(np.uint8)
    nib = nib.reshape(OUT_F, IN_F)  # n[o, i]
    lut = np.arange(16, dtype=np.float32).astype(ml_dtypes.float8_e4m3)
    nfp8 = lut[nib]  # [8192, 8192] fp8, exact

    xb = x.astype(ml_dtypes.bfloat16)
    xf = xb.astype(np.float32)
    # xt_host[p, r, t] = x_bf16[t, 128r + p]
    xt_host = np.ascontiguousarray(xb.T.reshape(NCHUNK, 128, TOKENS).transpose(1, 0, 2))
    # xsum_g[t] (with bf16-rounded x, matching the matmul operand)
    xsum = xf.reshape(TOKENS, NG, GROUP).sum(axis=2)  # [t, g]
    cr_host = np.concatenate(
        [xsum.T, np.ones((1, TOKENS), dtype=np.float32)], axis=0
    ).astype(np.float32)  # [33, 64]

    in_maps = []
    for k in range(NCORES):
        osl = slice(OC * k, OC * (k + 1))
        nk = nfp8[osl]  # [1024, 8192]
        # w8_host[p, r, o] = n[o, 128r + p]
        w8_host = np.ascontiguousarray(nk.T.reshape(NCHUNK, 128, OC).transpose(1, 0, 2))
        sck = scales[osl]  # [1024, 32]
        sc_host = np.ascontiguousarray(sck.reshape(NOT, 128, NG).transpose(1, 0, 2))
        uk = np.zeros_like(sck)
        uk[:, 1:] = sck[:, :-1] / sck[:, 1:]
        ur_host = np.ascontiguousarray(uk.reshape(NOT, 128, NG).transpose(1, 0, 2))
        s31 = sck[:, NG - 1]  # [1024]
        cl_host = np.empty((NG + 1, OC), dtype=np.float32)
        cl_host[:NG] = (-8.0 * sck / s31[:, None]).T
        cl_host[NG] = bias[osl] / s31
        in_maps.append({
            names["w8"]: w8_host,
            names["xt"]: xt_host,
            names["sc"]: sc_host.astype(np.float32),
            names["ur"]: ur_host.astype(np.float32),
            names["cl"]: cl_host,
            names["cr"]: cr_host,
        })
    return in_maps


def kernel(x, weight_packed, scales, bias):
    from concourse.bass_utils import run_bass_kernel_spmd

    nc, names = _build_nc()
    in_maps = _host_prep(x, weight_packed, scales, bias)
    res = run_bass_kernel_spmd(nc, in_maps, core_ids=list(range(NCORES)))
    outs = [res.results[k][names["outT"]] for k in range(NCORES)]  # [1024, 64] each
    out = np.concatenate([o.T for o in outs], axis=1)  # [64, 8192]
    return np.ascontiguousarray(out.astype(np.float32))



# revision 10
# speedup vs baseline: 1.0449x; 1.0449x over previous
"""Int4-packed linear (group-quantized, 256-group) on 8 Trainium2 cores.

Column-parallel: each core owns 1024 of 8192 out_features.

Math per core (out^T orientation, o on partitions):
  out[t, o] = sum_g s[o,g] * R_g[o,t] - 8*sum_g s[o,g]*xsum_g[t] + bias[o]
  R_g[o,t]  = sum_{i in g} q[o,i] * x[t,i]        (q in 0..15)

Weights ship as fp8e4m3 nibble planes (exact small integers), x as bf16.
Group partials accumulate in PSUM slices; -8 offset + bias ride a tiny fp32
correction matmul into group 31's slice (pre-divided by bf16(s[:,31]) so the
on-chip bf16 scale multiply restores it exactly).

Combine (per o-tile) is a 3-engine pipeline instead of a DVE scan:
  ACT    : PSUM fp32 [g, t] -> SBUF bf16 [t, g] (transposing copy drain)
  DVE    : sp = rsb * s_bcast   (g innermost stride-1 -> 2x bf16 mode)
  GPSIMD : tree levels 1-2 (32 -> 8 groups)
  DVE    : tree levels 3..5 (8 -> 1), fp32
Weight DMA is split into 8 per-o-tile blocks so each o-tile's matmul chain
starts as soon as its own 1.05MB block lands.
"""

import sys

import numpy as np
import ml_dtypes

sys.path.insert(0, "/opt/trn_rl_repo")

import concourse.bass as bass  # noqa: E402
import concourse.mybir as mybir  # noqa: E402
import concourse.tile as tile  # noqa: E402
from concourse import bacc  # noqa: E402

NCORES = 8
TOKENS = 64
IN_F = 8192
OUT_F = 8192
GROUP = 256
OC = OUT_F // NCORES  # 1024 out-features per core
NCHUNK = IN_F // 128  # 64 K-chunks of 128
NG = IN_F // GROUP  # 32 groups
NOT = OC // 128  # 8 o-tiles per core

_cache = {}

ADD = mybir.AluOpType.add
MULT = mybir.AluOpType.mult


def _build_nc():
    if "nc" in _cache:
        return _cache["nc"], _cache["names"]

    f32 = mybir.dt.float32
    bf16 = mybir.dt.bfloat16
    fp8 = mybir.dt.float8e4
    nc = bacc.Bacc(None, target_bir_lowering=False, debug=False)
    with tile.TileContext(nc) as tc:
        with tc.tile_pool(name="dram", bufs=1, space="DRAM") as dram:
            w8 = dram.tile([128, NOT, NCHUNK, 128], fp8, kind="ExternalInput")
            xt = dram.tile([128, NCHUNK, TOKENS], bf16, kind="ExternalInput")
            s2 = dram.tile([128, NOT, NG], bf16, kind="ExternalInput")
            cl = dram.tile([NG + 1, OC], f32, kind="ExternalInput")
            cr = dram.tile([NG + 1, TOKENS], f32, kind="ExternalInput")
            outT = dram.tile([OC, TOKENS], f32, kind="ExternalOutput")

            with (
                tc.tile_pool(name="wsb", bufs=1) as wsb,
                tc.tile_pool(name="xsb", bufs=1) as xsb,
                tc.tile_pool(name="small", bufs=1) as small,
                tc.tile_pool(name="rsb", bufs=2) as rsbp,
                tc.tile_pool(name="spp", bufs=2) as spp,
                tc.tile_pool(name="t16", bufs=2) as t16p,
                tc.tile_pool(name="t8", bufs=2) as t8p,
                tc.tile_pool(name="t4", bufs=2) as t4p,
                tc.tile_pool(name="t2", bufs=2) as t2p,
                tc.tile_pool(name="yout", bufs=3) as ypool,
                tc.tile_pool(name="ps", bufs=2, space="PSUM") as ps,
            ):
                w_all = wsb.tile([128, NOT, NCHUNK, 128], fp8)
                x_all = xsb.tile([128, NCHUNK, TOKENS], bf16)
                s2_all = small.tile([128, NOT, NG], bf16)
                cl_sb = small.tile([NG + 1, OC], f32, tag="cl")
                cr_sb = small.tile([NG + 1, TOKENS], f32, tag="cr")

                # x + small operands ride the Activation HWDGE queue so the
                # weight blocks (SP queue) start transferring immediately
                nc.scalar.dma_start(out=x_all[:], in_=xt[:])
                nc.scalar.dma_start(out=s2_all[:], in_=s2[:])
                nc.scalar.dma_start(out=cl_sb[:], in_=cl[:])
                nc.scalar.dma_start(out=cr_sb[:], in_=cr[:])
                for b in range(NOT):
                    nc.sync.dma_start(out=w_all[:, b, :, :], in_=w8[:, b, :, :])

                pend = None  # (t2, ot) whose tail tree is not yet emitted

                def emit_tail(t2, ot):
                    osl = slice(ot * 128, (ot + 1) * 128)
                    y = ypool.tile([128, TOKENS], f32)
                    nc.vector.tensor_tensor(
                        out=y[:], in0=t2[:, :, 0], in1=t2[:, :, 1], op=ADD,
                    )
                    nc.scalar.dma_start(out=outT[osl, :], in_=y[:])

                for ot in range(NOT):
                    osl = slice(ot * 128, (ot + 1) * 128)
                    r_ps = ps.tile([128, NG, TOKENS], f32)
                    for g in range(NG):
                        nc.tensor.matmul(
                            r_ps[:, g, :],
                            lhsT=w_all[:, ot, 2 * g, :],
                            rhs=x_all[:, 2 * g, :],
                            start=True,
                            stop=False,
                        )
                        nc.tensor.matmul(
                            r_ps[:, g, :],
                            lhsT=w_all[:, ot, 2 * g + 1, :],
                            rhs=x_all[:, 2 * g + 1, :],
                            start=False,
                            stop=(g != NG - 1),
                        )
                    # -8 offset + bias correction, pre-divided by bf16(s[:,31])
                    nc.tensor.matmul(
                        r_ps[:, NG - 1, :],
                        lhsT=cl_sb[:, osl],
                        rhs=cr_sb[:],
                        start=False,
                        stop=True,
                    )

                    # ACT: transposing drain PSUM [g,t] -> SBUF bf16 [t,g] so
                    # the mult's innermost dim is g with stride 1 (2x mode)
                    rsb = rsbp.tile([128, TOKENS, NG], bf16)
                    rp_t = bass.AP(
                        tensor=r_ps.tensor,
                        offset=r_ps.offset,
                        ap=[r_ps.ap[0], [1, TOKENS], [TOKENS, NG]],
                    )
                    nc.scalar.copy(out=rsb[:], in_=rp_t)

                    # DVE: sp[t,g] = rsb[t,g] * s[g]
                    sp = spp.tile([128, TOKENS, NG], bf16)
                    s2_ot = s2_all[:, ot, :]
                    s_ap = bass.AP(
                        tensor=s2_ot.tensor,
                        offset=s2_ot.offset,
                        ap=[s2_ot.ap[0], [0, TOKENS], [1, NG]],
                    )
                    nc.vector.scalar_tensor_tensor(
                        out=sp[:], in0=rsb[:], scalar=1.0, in1=s_ap,
                        op0=MULT, op1=MULT,
                    )

                    # GPSIMD: tree levels 1-2 (32 -> 8 groups)
                    t16 = t16p.tile([128, TOKENS, 16], f32)
                    nc.gpsimd.tensor_tensor(
                        out=t16[:], in0=sp[:, :, 0:16], in1=sp[:, :, 16:32],
                        op=ADD,
                    )
                    t8 = t8p.tile([128, TOKENS, 8], f32)
                    nc.gpsimd.tensor_tensor(
                        out=t8[:], in0=t16[:, :, 0:8], in1=t16[:, :, 8:16],
                        op=ADD,
                    )

                    # DVE: tree levels 3..4 for this tile; the level-5 tail of
                    # the PREVIOUS tile goes after, so DVE never waits on the
                    # gpsimd levels of the current tile
                    if pend is not None:
                        emit_tail(*pend)
                    t4 = t4p.tile([128, TOKENS, 4], f32)
                    nc.vector.tensor_tensor(
                        out=t4[:], in0=t8[:, :, 0:4], in1=t8[:, :, 4:8], op=ADD,
                    )
                    t2 = t2p.tile([128, TOKENS, 2], f32)
                    nc.vector.tensor_tensor(
                        out=t2[:], in0=t4[:, :, 0:2], in1=t4[:, :, 2:4], op=ADD,
                    )
                    pend = (t2, ot)

                emit_tail(*pend)

    nc.compile()
    names = dict(w8=w8.name, xt=xt.name, s2=s2.name, cl=cl.name, cr=cr.name,
                 outT=outT.name)
    _cache["nc"] = nc
    _cache["names"] = names
    return nc, names


def _host_prep(x, weight_packed, scales, bias):
    """Build the 8 per-core input maps."""
    _, names = _build_nc()

    bf16 = ml_dtypes.bfloat16
    wp = np.ascontiguousarray(weight_packed).view(np.uint32)  # [8192, 1024]
    shifts = (np.arange(8, dtype=np.uint32) * 4)[None, None, :]
    nib = ((wp[:, :, None] >> shifts) & np.uint32(0xF)).astype(np.uint8)
    nib = nib.reshape(OUT_F, IN_F)  # n[o, i]
    lut = np.arange(16, dtype=np.float32).astype(ml_dtypes.float8_e4m3)
    nfp8 = lut[nib]  # [8192, 8192] fp8, exact

    xb = x.astype(bf16)
    xf = xb.astype(np.float32)
    # xt_host[p, c, t] = x_bf16[t, 128c + p]
    xt_host = np.ascontiguousarray(xb.T.reshape(NCHUNK, 128, TOKENS).transpose(1, 0, 2))
    # xsum_g[t] (with bf16-rounded x, matching the matmul operand)
    xsum = xf.reshape(TOKENS, NG, GROUP).sum(axis=2)  # [t, g]
    cr_host = np.concatenate(
        [xsum.T, np.ones((1, TOKENS), dtype=np.float32)], axis=0
    ).astype(np.float32)  # [33, 64]

    in_maps = []
    for k in range(NCORES):
        osl = slice(OC * k, OC * (k + 1))
        nk = nfp8[osl]  # [1024, 8192]
        # w8_host[p, b, c, j] = n[128b + j, 128c + p]
        w8_host = np.ascontiguousarray(
            nk.reshape(NOT, 128, NCHUNK, 128).transpose(3, 0, 2, 1)
        )
        sck = np.asarray(scales[osl], dtype=np.float32)  # [1024, 32]
        sb = sck.astype(bf16)  # bf16 scales used on-chip
        s31b = sb[:, NG - 1].astype(np.float32)  # bf16-rounded s31
        # s2_host[p, ot, g] = bf16(s[128*ot + p, g])
        s2_host = np.ascontiguousarray(
            sb.reshape(NOT, 128, NG).transpose(1, 0, 2)
        )  # [128, 8, 32] bf16
        cl_host = np.empty((NG + 1, OC), dtype=np.float32)
        cl_host[:NG] = (-8.0 * sck / s31b[:, None]).T
        cl_host[NG] = np.asarray(bias[osl], dtype=np.float32) / s31b
        in_maps.append({
            names["w8"]: w8_host,
            names["xt"]: xt_host,
            names["s2"]: s2_host,
            names["cl"]: cl_host,
            names["cr"]: cr_host,
        })
    return in_maps


def kernel(x, weight_packed, scales, bias):
    from concourse.bass_utils import run_bass_kernel_spmd

    nc, names = _build_nc()
    in_maps = _host_prep(x, weight_packed, scales, bias)
    res = run_bass_kernel_spmd(nc, in_maps, core_ids=list(range(NCORES)))
    outs = [res.results[k][names["outT"]] for k in range(NCORES)]  # [1024, 64] each
    out = np.concatenate([o.T for o in outs], axis=1)  # [64, 8192]
    return np.ascontiguousarray(out.astype(np.float32))


# revision 12
# speedup vs baseline: 1.1810x; 1.1302x over previous
"""Int4-packed linear (group-quantized, 256-group) on 8 Trainium2 cores.

Column-parallel: each core owns 1024 of 8192 out_features.

Math per core (out^T orientation, o on partitions):
  out[t, o] = sum_g s[o,g] * R_g[o,t] - 8*sum_g s[o,g]*xsum_g[t] + bias[o]
  R_g[o,t]  = sum_{i in g} q[o,i] * x[t,i]        (q in 0..15)

Weights ship as fp8e4m3 nibble planes (exact small integers), x as bf16.
Group partials accumulate in PSUM slices; -8 offset + bias ride a tiny fp32
correction matmul into group 31's slice (pre-divided by bf16(s[:,31]) so the
on-chip bf16 scale multiply restores it exactly).

Combine (per o-tile) is a DVE+GPSIMD pipeline in [g, t] layout (tree
halves contiguous -> 2x bf16 DVE mode):
  DVE    : sp[g,t] = r_ps[g,t] * s[g]   (PSUM read, 1x, bf16 out)
  GPSIMD : tree levels 1 and 3 (bf16)
  DVE    : tree levels 2, 4, 5 (bf16 2x; last two fp32)
Weight DMA is split into 8 per-o-tile blocks so each o-tile's matmul chain
starts as soon as its own 1.05MB block lands; y accumulates in SBUF and
ships as one 2KB-per-partition-row DMA at the end.
"""

import sys

import numpy as np
import ml_dtypes

sys.path.insert(0, "/opt/trn_rl_repo")

import concourse.bass as bass  # noqa: E402
import concourse.mybir as mybir  # noqa: E402
import concourse.tile as tile  # noqa: E402
from concourse import bacc  # noqa: E402

NCORES = 8
TOKENS = 64
IN_F = 8192
OUT_F = 8192
GROUP = 256
OC = OUT_F // NCORES  # 1024 out-features per core
NCHUNK = IN_F // 128  # 64 K-chunks of 128
NG = IN_F // GROUP  # 32 groups
NOT = OC // 128  # 8 o-tiles per core

_cache = {}

ADD = mybir.AluOpType.add
MULT = mybir.AluOpType.mult


def _build_nc():
    if "nc" in _cache:
        return _cache["nc"], _cache["names"]

    f32 = mybir.dt.float32
    bf16 = mybir.dt.bfloat16
    fp8 = mybir.dt.float8e4
    nc = bacc.Bacc(None, target_bir_lowering=False, debug=False)
    with tile.TileContext(nc) as tc:
        with tc.tile_pool(name="dram", bufs=1, space="DRAM") as dram:
            w8 = dram.tile([128, NOT, NCHUNK, 128], fp8, kind="ExternalInput")
            xt = dram.tile([128, NCHUNK, TOKENS], bf16, kind="ExternalInput")
            s2 = dram.tile([128, NOT, NG], bf16, kind="ExternalInput")
            cl = dram.tile([NG + 1, OC], f32, kind="ExternalInput")
            cr = dram.tile([NG + 1, TOKENS], f32, kind="ExternalInput")
            outT = dram.tile([128, NOT, TOKENS], f32, kind="ExternalOutput")

            with (
                tc.tile_pool(name="wsb", bufs=1) as wsb,
                tc.tile_pool(name="xsb", bufs=1) as xsb,
                tc.tile_pool(name="small", bufs=1) as small,
                tc.tile_pool(name="spp", bufs=2) as spp,
                tc.tile_pool(name="t16", bufs=2) as t16p,
                tc.tile_pool(name="t8", bufs=2) as t8p,
                tc.tile_pool(name="t4", bufs=2) as t4p,
                tc.tile_pool(name="t2", bufs=2) as t2p,
                tc.tile_pool(name="yout", bufs=1) as ypool,
                tc.tile_pool(name="ps", bufs=2, space="PSUM") as ps,
            ):
                w_all = wsb.tile([128, NOT, NCHUNK, 128], fp8)
                x_all = xsb.tile([128, NCHUNK, TOKENS], bf16)
                s2_all = small.tile([128, NOT, NG], bf16)
                cl_sb = small.tile([NG + 1, OC], f32, tag="cl")
                cr_sb = small.tile([NG + 1, TOKENS], f32, tag="cr")
                y_all = ypool.tile([128, NOT, TOKENS], f32)

                # x + small operands ride the Activation HWDGE queue so the
                # weight blocks (SP queue) start transferring immediately
                nc.scalar.dma_start(out=x_all[:], in_=xt[:])
                nc.scalar.dma_start(out=s2_all[:], in_=s2[:])
                nc.scalar.dma_start(out=cl_sb[:], in_=cl[:])
                nc.scalar.dma_start(out=cr_sb[:], in_=cr[:])
                for b in range(NOT):
                    nc.sync.dma_start(out=w_all[:, b, :, :], in_=w8[:, b, :, :])

                sps, t16s, t8s, t4s, t2s = {}, {}, {}, {}, {}

                def chain_and_mult(ot):
                    osl = slice(ot * 128, (ot + 1) * 128)
                    r_ps = ps.tile([128, NG, TOKENS], f32)
                    for g in range(NG):
                        nc.tensor.matmul(
                            r_ps[:, g, :],
                            lhsT=w_all[:, ot, 2 * g, :],
                            rhs=x_all[:, 2 * g, :],
                            start=True,
                            stop=False,
                        )
                        nc.tensor.matmul(
                            r_ps[:, g, :],
                            lhsT=w_all[:, ot, 2 * g + 1, :],
                            rhs=x_all[:, 2 * g + 1, :],
                            start=False,
                            stop=(g != NG - 1),
                        )
                    # -8 offset + bias correction, pre-divided by bf16(s[:,31])
                    nc.tensor.matmul(
                        r_ps[:, NG - 1, :],
                        lhsT=cl_sb[:, osl],
                        rhs=cr_sb[:],
                        start=False,
                        stop=True,
                    )
                    # DVE: sp[g,t] = r_ps[g,t] * s[g]  (PSUM read, 1x)
                    sp = spp.tile([128, NG, TOKENS], bf16)
                    s2_ot = s2_all[:, ot, :]
                    s_ap = bass.AP(
                        tensor=s2_ot.tensor,
                        offset=s2_ot.offset,
                        ap=[s2_ot.ap[0], [1, NG], [0, TOKENS]],
                    )
                    nc.vector.tensor_tensor(
                        out=sp[:], in0=r_ps[:], in1=s_ap, op=MULT)
                    sps[ot] = sp

                def l1(ot):  # gpsimd 32 -> 16
                    sp = sps.pop(ot)
                    t16 = t16p.tile([128, 16, TOKENS], bf16)
                    nc.gpsimd.tensor_tensor(
                        out=t16[:], in0=sp[:, 0:16, :], in1=sp[:, 16:32, :],
                        op=ADD)
                    t16s[ot] = t16

                def l2(ot):  # DVE 16 -> 8 (bf16 2x)
                    t16 = t16s.pop(ot)
                    t8 = t8p.tile([128, 8, TOKENS], bf16)
                    nc.vector.tensor_tensor(
                        out=t8[:], in0=t16[:, 0:8, :], in1=t16[:, 8:16, :],
                        op=ADD)
                    t8s[ot] = t8

                def l3(ot):  # gpsimd 8 -> 4
                    t8 = t8s.pop(ot)
                    t4 = t4p.tile([128, 4, TOKENS], bf16)
                    nc.gpsimd.tensor_tensor(
                        out=t4[:], in0=t8[:, 0:4, :], in1=t8[:, 4:8, :],
                        op=ADD)
                    t4s[ot] = t4

                def l45(ot):  # DVE 4 -> 1, fp32
                    t4 = t4s.pop(ot)
                    t2 = t2p.tile([128, 2, TOKENS], f32)
                    nc.vector.tensor_tensor(
                        out=t2[:], in0=t4[:, 0:2, :], in1=t4[:, 2:4, :],
                        op=ADD)
                    nc.vector.tensor_tensor(
                        out=y_all[:, ot, :], in0=t2[:, 0, :], in1=t2[:, 1, :],
                        op=ADD)

                for ot in range(NOT):
                    chain_and_mult(ot)
                    l1(ot)
                    if ot >= 1:
                        l2(ot - 1)
                        l3(ot - 1)
                    if ot >= 2:
                        l45(ot - 2)
                l2(NOT - 1)
                l3(NOT - 1)
                l45(NOT - 2)
                l45(NOT - 1)

                nc.scalar.dma_start(out=outT[:], in_=y_all[:])

    nc.compile()
    names = dict(w8=w8.name, xt=xt.name, s2=s2.name, cl=cl.name, cr=cr.name,
                 outT=outT.name)
    _cache["nc"] = nc
    _cache["names"] = names
    return nc, names


def _host_prep(x, weight_packed, scales, bias):
    """Build the 8 per-core input maps."""
    _, names = _build_nc()

    bf16 = ml_dtypes.bfloat16
    wp = np.ascontiguousarray(weight_packed).view(np.uint32)  # [8192, 1024]
    shifts = (np.arange(8, dtype=np.uint32) * 4)[None, None, :]
    nib = ((wp[:, :, None] >> shifts) & np.uint32(0xF)).astype(np.uint8)
    nib = nib.reshape(OUT_F, IN_F)  # n[o, i]
    lut = np.arange(16, dtype=np.float32).astype(ml_dtypes.float8_e4m3)
    nfp8 = lut[nib]  # [8192, 8192] fp8, exact

    xb = x.astype(bf16)
    xf = xb.astype(np.float32)
    # xt_host[p, c, t] = x_bf16[t, 128c + p]
    xt_host = np.ascontiguousarray(xb.T.reshape(NCHUNK, 128, TOKENS).transpose(1, 0, 2))
    # xsum_g[t] (with bf16-rounded x, matching the matmul operand)
    xsum = xf.reshape(TOKENS, NG, GROUP).sum(axis=2)  # [t, g]
    cr_host = np.concatenate(
        [xsum.T, np.ones((1, TOKENS), dtype=np.float32)], axis=0
    ).astype(np.float32)  # [33, 64]

    in_maps = []
    for k in range(NCORES):
        osl = slice(OC * k, OC * (k + 1))
        nk = nfp8[osl]  # [1024, 8192]
        # w8_host[p, b, c, j] = n[128b + j, 128c + p]
        w8_host = np.ascontiguousarray(
            nk.reshape(NOT, 128, NCHUNK, 128).transpose(3, 0, 2, 1)
        )
        sck = np.asarray(scales[osl], dtype=np.float32)  # [1024, 32]
        sb = sck.astype(bf16)  # bf16 scales used on-chip
        s31b = sb[:, NG - 1].astype(np.float32)  # bf16-rounded s31
        # s2_host[p, ot, g] = bf16(s[128*ot + p, g])
        s2_host = np.ascontiguousarray(
            sb.reshape(NOT, 128, NG).transpose(1, 0, 2)
        )  # [128, 8, 32] bf16
        cl_host = np.empty((NG + 1, OC), dtype=np.float32)
        cl_host[:NG] = (-8.0 * sck / s31b[:, None]).T
        cl_host[NG] = np.asarray(bias[osl], dtype=np.float32) / s31b
        in_maps.append({
            names["w8"]: w8_host,
            names["xt"]: xt_host,
            names["s2"]: s2_host,
            names["cl"]: cl_host,
            names["cr"]: cr_host,
        })
    return in_maps


def kernel(x, weight_packed, scales, bias):
    from concourse.bass_utils import run_bass_kernel_spmd

    nc, names = _build_nc()
    in_maps = _host_prep(x, weight_packed, scales, bias)
    res = run_bass_kernel_spmd(nc, in_maps, core_ids=list(range(NCORES)))
    # outT[p, ot, t] -> out[t, k*1024 + ot*128 + p]
    outs = [
        np.asarray(res.results[k][names["outT"]]).transpose(1, 0, 2).reshape(OC, TOKENS)
        for k in range(NCORES)
    ]
    out = np.concatenate([o.T for o in outs], axis=1)  # [64, 8192]
    return np.ascontiguousarray(out.astype(np.float32))
